# revision 30
# baseline (speedup 1.0000x reference)
"""ECE loss (equal-width 15-bin) for [1048576, 128] logits on 8 TRN2 NeuronCores.

Strategy (data-parallel over rows, per the sharding hint):
  Device, per core (N/8 = 131072 rows):
    - stream [128 partitions, G rows, 128 classes] supertiles of y_pred
      (warm-up/cool-down schedule: small tiles at both ends so compute
      starts early and the post-stream tail is short)
    - DVE:   grouped reduce_max over classes -> per-row max m (exact f32)
    - sum path on the otherwise-idle PE: per 128-row chunk,
      PE-transpose x into PSUM (exact data movement), ACT exps two
      PSUM banks at a time PSUM->SBUF as bf16 (class-major), then a PE
      matmul with a ones vector contracts over the class partitions:
      U[row] = sum_c exp(x[row, c]).  PE accumulates in f32; the only
      precision loss is bf16 rounding of each exp (~0.2% per element,
      averaging out over 128 -> U accurate to ~1e-3, far inside the
      binning tolerance).  Unshifted exp is safe: |x| <= ~6.5.
    - the last (1-PE_FRAC) rows of each full supertile take the
      row-major path instead: batched f32 ACT exp + DVE grouped
      reduce_sum -- balances PE (~199ns/chunk) vs DVE (~133ns/chunk
      marginal) so no engine exceeds the HBM stream pace.
    - outputs m, u (natural row order) -- a 512MB -> 1MB reduction
  Host:
    conf = exp(m)/U  (== max softmax);  acc = (y_pred[r, y_true[r]] == m)
    (the row max is an exact element of the row, so float equality
    reproduces argmax == label up to exact-tie rows), then the 15-bin
    equal-width histogram and the final ECE reduction as in the reference.

Measured per core: engines ~150-175us busy (DVE max 142 + sums, ACT exp
~150, PE transposes+matmuls ~170), under the contended HBM stream of
64MB at ~290-330GB/s/core (8 cores saturate ~2.6TB/s chip HBM); the
kernel is stream-bound end-to-end, slowest core ~220-245us total.
"""

import numpy as np

import concourse.bacc as bacc
import concourse.bass as bass
import concourse.tile as tile
from concourse import masks, mybir
from concourse.bass_utils import run_bass_kernel_spmd

N_CORES = 8
N = 1048576
C = 128
N_SHARD = N // N_CORES  # 131072
P = 128                 # SBUF partitions
T = N_SHARD // P        # 1024 rows handled per partition
N_BINS = 15

PE_FRAC = 26 / 32       # fraction of each full supertile's rows on the PE path
FLUSH_EVERY = 8

# warm-up: small leading supertiles so compute starts early; cool-down:
# small trailing tiles so the post-stream compute tail is short.
def _schedule():
    gs = [4] * 2 + [8] * 7 + [16] * 4 + [32] * 27 + [16, 8, 4, 4]
    assert sum(gs) == T
    sched = []
    t0 = 0
    for g in gs:
        sched.append((t0, g))
        t0 += g
    return sched

SCHED = _schedule()

_CACHE: dict = {}


def _build_bass():
    nc = bacc.Bacc(None, target_bir_lowering=False)
    x = nc.dram_tensor("x", [N_SHARD, C], mybir.dt.float32, kind="ExternalInput")
    ident_in = nc.dram_tensor("ident", [P, P], mybir.dt.float32, kind="ExternalInput")
    m_out = nc.dram_tensor("m_out", [N_SHARD], mybir.dt.float32, kind="ExternalOutput")
    u_out = nc.dram_tensor("u_out", [N_SHARD], mybir.dt.float32, kind="ExternalOutput")

    # row r = p*T + t lives at [p, t]; per-partition runs in DRAM stay contiguous
    xv = x[:, :].rearrange("(p t) c -> p t c", p=P)
    mv = m_out[:].rearrange("(p t) -> p t", p=P)
    uv = u_out[:].rearrange("(p t) -> p t", p=P)

    with tile.TileContext(nc) as tc:
        with (
            tc.tile_pool(name="xin", bufs=11) as xin_pool,
            tc.tile_pool(name="ett", bufs=4) as ett_pool,
            tc.tile_pool(name="erow", bufs=2) as erow_pool,
            tc.tile_pool(name="singles", bufs=1) as singles,
            tc.tile_pool(name="stats", bufs=1) as stats_pool,
            tc.tile_pool(name="pst", bufs=3, space=bass.MemorySpace.PSUM) as pst_pool,
            tc.tile_pool(name="pu", bufs=2, space=bass.MemorySpace.PSUM) as pu_pool,
        ):
            m_all = stats_pool.tile([P, T], mybir.dt.float32)
            u_all = stats_pool.tile([P, T], mybir.dt.float32)

            identity = singles.tile([P, P], mybir.dt.float32)
            ones = singles.tile([P, 1], mybir.dt.bfloat16)

            prev_pu = None  # (pu_tile, t0, g) awaiting copy into u_all
            flushed = 0
            for si, (t0, g) in enumerate(SCHED):
                xt = xin_pool.tile([P, g, C], mybir.dt.float32, tag="xt")
                nc.sync.dma_start(out=xt[:], in_=xv[:, t0 : t0 + g, :])
                if si == 0:
                    # after the first x tile so compute starts ASAP
                    nc.sync.dma_start(out=identity[:], in_=ident_in[:, :])
                    nc.vector.memset(ones[:], 1.0)
                nc.vector.reduce_max(
                    out=m_all[:, t0 : t0 + g],
                    in_=xt[:],
                    axis=mybir.AxisListType.X,
                )
                # drain the previous supertile's U psum on the lighter-loaded
                # Scalar engine, now that its matmuls are long done
                if prev_pu is not None:
                    pu_p, pt0, pg = prev_pu
                    nc.scalar.copy(out=u_all[:, pt0 : pt0 + pg], in_=pu_p[:, 0:pg])
                    prev_pu = None

                npe = g if g < 32 else int(g * PE_FRAC)
                pu = pu_pool.tile([P, 32], mybir.dt.float32, tag="pu")
                # PE path: per up-to-8-chunk group (2 PSUM banks), transposes
                # -> one batched exp(bf16) -> matmuls (deferred one group so
                # the in-order PE queue never waits on the just-issued exp)
                pending = None  # (ett_tile, base_chunk, k) with deferred matmuls
                for base in range(0, npe, 8):
                    k = min(8, npe - base)
                    pst = pst_pool.tile([P, 1024], mybir.dt.float32, tag="pst")
                    for j in range(k):
                        nc.tensor.transpose(
                            pst[:, j * P : (j + 1) * P], xt[:, base + j, :], identity[:]
                        )
                    ett = ett_pool.tile([P, 1024], mybir.dt.bfloat16, tag="ett")
                    nc.scalar.activation(
                        out=ett[:, 0 : k * P],
                        in_=pst[:, 0 : k * P],
                        func=mybir.ActivationFunctionType.Exp,
                    )
                    if pending is not None:
                        pett, pbase, pk = pending
                        for j in range(pk):
                            nc.tensor.matmul(
                                out=pu[:, pbase + j : pbase + j + 1],
                                lhsT=pett[:, j * P : (j + 1) * P],
                                rhs=ones[:],
                            )
                    pending = (ett, base, k)
                if pending is not None:
                    pett, pbase, pk = pending
                    for j in range(pk):
                        nc.tensor.matmul(
                            out=pu[:, pbase + j : pbase + j + 1],
                            lhsT=pett[:, j * P : (j + 1) * P],
                            rhs=ones[:],
                        )
                prev_pu = (pu, t0, npe)

                # row-major fallback path for the remaining rows (if any)
                if npe < g:
                    er = erow_pool.tile([P, g - npe, C], mybir.dt.float32, tag="er")
                    nc.scalar.activation(
                        out=er[:],
                        in_=xt[:, npe:g, :],
                        func=mybir.ActivationFunctionType.Exp,
                    )
                    nc.vector.reduce_sum(
                        out=u_all[:, t0 + npe : t0 + g],
                        in_=er[:],
                        axis=mybir.AxisListType.X,
                    )

                # flush periodically, and after every cool-down tile so the
                # final post-stream flush carries almost nothing. Flushes go
                # out on the idle GPSIMD queue: on Sync they sit in program
                # order waiting for compute and dam up the input-DMA issues
                # behind them (measured: input issue bursts at exactly the
                # flush cadence, starving the DMA engines in between).
                if (si % FLUSH_EVERY == FLUSH_EVERY - 1 or si >= len(SCHED) - 5) and (
                    si != len(SCHED) - 1
                ):
                    done = t0  # u_all/m_all final through the previous supertile
                    if done > flushed:
                        nc.gpsimd.dma_start(
                            out=mv[:, flushed:done], in_=m_all[:, flushed:done]
                        )
                        nc.gpsimd.dma_start(
                            out=uv[:, flushed:done], in_=u_all[:, flushed:done]
                        )
                        flushed = done
            if prev_pu is not None:
                pu_p, pt0, pg = prev_pu
                nc.scalar.copy(out=u_all[:, pt0 : pt0 + pg], in_=pu_p[:, 0:pg])
            nc.gpsimd.dma_start(out=mv[:, flushed:T], in_=m_all[:, flushed:T])
            nc.gpsimd.dma_start(out=uv[:, flushed:T], in_=u_all[:, flushed:T])
    nc.finalize()
    return nc


def run_device(y_pred: np.ndarray, **spmd_kwargs):
    """Run the bass kernel on 8 cores; returns (m, U) each [N] f32 plus results obj."""
    if "nc" not in _CACHE:
        _CACHE["nc"] = _build_bass()
    nc = _CACHE["nc"]
    ident = np.eye(P, dtype=np.float32)
    in_maps = [
        {"x": y_pred[c * N_SHARD : (c + 1) * N_SHARD], "ident": ident}
        for c in range(N_CORES)
    ]
    res = run_bass_kernel_spmd(nc, in_maps, core_ids=list(range(N_CORES)), **spmd_kwargs)
    m = np.concatenate([r["m_out"] for r in res.results])
    u = np.concatenate([r["u_out"] for r in res.results])
    return m, u, res


def finish_host(y_pred, y_true, m, u) -> np.ndarray:
    xl = y_pred[np.arange(N), np.asarray(y_true, dtype=np.int64)]
    conf = np.exp(m.astype(np.float64)) / u.astype(np.float64)
    acc = (xl == m).astype(np.float64)
    bin_idx = np.clip(np.ceil(conf * N_BINS).astype(np.int64) - 1, 0, N_BINS - 1)
    cnt = np.bincount(bin_idx, minlength=N_BINS).astype(np.float64)
    conf_sum = np.bincount(bin_idx, weights=conf, minlength=N_BINS)
    acc_sum = np.bincount(bin_idx, weights=acc, minlength=N_BINS)
    safe = np.where(cnt > 0, cnt, 1.0)
    per_bin = np.where(cnt > 0, np.abs(conf_sum / safe - acc_sum / safe) * (cnt / N), 0.0)
    return np.array([per_bin.sum()], dtype=np.float32)


def kernel(y_pred: np.ndarray, y_true: np.ndarray) -> np.ndarray:
    y_pred = np.ascontiguousarray(np.asarray(y_pred, dtype=np.float32))
    m, u, _ = run_device(y_pred)
    return finish_host(y_pred, y_true, m, u)
